# revision 30
# baseline (speedup 1.0000x reference)
"""Distributed Trainium2 kernel for the Koopman-operator problem.

Math (from the reference):
    X  = x.reshape(64, T)                 # T = 524288, pure row-major view
    M  = L @ L.T                          # 128x128;  M11, M21, M22 are 64x64 blocks
    B  = 2*(M11 + M22 + R - R.T)          # (eps*I is ~1e-8, negligible vs O(30) entries)
    A  = inv(B) @ M21
    out = (A @ X).reshape(-1, 64)

Distribution: column-shard X across 8 cores (65536 cols each) -- fully
data-parallel, zero collectives.  L and R are replicated; every core
redundantly computes inv(B/2) on device.

Inverse: Frobenius-scaled Newton-Schulz by repeated squaring.
E = I - Bh Bh^T/s (s = ||Bh||_F^2) is symmetric with spectrum in
(0,1); inv(Bh) = (Bh^T/s) prod_j (I + E^(2^j)).  The F-chain
(F <- F@F, 21 steps) is the only serial recurrence; the (I + F_j)
factors all have spectrum in [1,2] (well conditioned -- this is why
this form is robust to hardware matmul rounding where the faster
Chebyshev-scaled Newton variant, whose intermediate factors pass
near singularity, is not), and are multiplied in an incremental
pairwise tree interleaved with the chain.  One fp32 Newton polish
squares the residual; measured end-to-end error ~4e-4 vs 2e-2.

I/O is fp16 (host casts, device computes in fp16->f32 PSUM): halves
HBM traffic, which is one of the three balanced rooflines here.  Per
core the shard is pre-stacked on host as (128, 32768): rows 0:64 hold
the first 32768 columns, rows 64:128 the next.  The stationary matrix
of the streaming matmul is blockdiag(At, At) (128x128 fp16), doubling
PE utilization.  At is produced directly in both partition halves by
running the final small matmul twice with different output partition
offsets (engines cannot copy across partitions; this avoids an
SBUF->SBUF DMA on the critical path).

Stream phase: whole fp16 shard is SBUF-resident (DMAs issued before
any compute, overlapping the inverse); the chain PSUM pools are
released and the stream takes all 8 PSUM banks as two (128,2048)
tiles; each is drained by a single cast-copy alternating between the
vector and scalar engines (the copy engines are the stream-phase
bottleneck at ~17 us); output DMAs go out every 2048 columns.
"""

import os
import sys

import numpy as np

for _p in ("/opt/trn_rl_repo", "/root/.axon_site/_ro/trn_rl_repo"):
    if _p not in sys.path and os.path.isdir(_p):
        sys.path.append(_p)

import concourse.bass as bass
import concourse.mybir as mybir
from concourse import bacc
from concourse.bass_utils import run_bass_kernel_spmd

from concourse.tile import TileContext

F32 = mybir.dt.float32
F16 = mybir.dt.float16

N = 64                   # state dim
N_CORES = 8
T_FULL = 524288          # columns of the reshaped X
T_CORE = T_FULL // N_CORES       # 65536 columns per core
T_HALF = T_CORE // 2             # 32768 -> free dim of the (128, .) shard

N_SQUARE = 21            # factors (I + E^(2^j)), j=0..20
# NOTE: f32r (replicated fp32) was tried for the small matmuls -- one HW
# instruction instead of fp32's two-pass split -- but its rounding on real
# hardware is far coarser than CoreSim models and broke the 2e-2 gate.

MM_COLS = 512            # matmul moving free dim (one PSUM bank, f32)
DMA_COLS = 4096          # input DMA chunk = 128 x 4096 x 2B = 1 MiB
OUT_COLS = 2048          # stream tile = 4 PSUM banks; output DMA = 0.5 MiB


def build_kernel(t_half=T_HALF):
    nc = bacc.Bacc()

    x_ext = nc.declare_dram_parameter("x", [128, t_half], F16, isOutput=False)
    # all small params packed into one tensor -> one SP DMA issue (~0.7us
    # each) instead of five ahead of the chain's first matmul
    pk_ext = nc.declare_dram_parameter("PK", [128, 384], F32, isOutput=False)
    out_ext = nc.declare_dram_parameter("out", [128, t_half], F16, isOutput=True)

    n_chunks = t_half // DMA_COLS

    with TileContext(nc) as tc:
        with (
            tc.tile_pool(name="const", bufs=1) as cpool,
            tc.tile_pool(name="small", bufs=2) as spool,
            tc.tile_pool(name="uch", bufs=1) as upool,
            tc.tile_pool(name="xin", bufs=1) as xpool,
            tc.tile_pool(name="yout", bufs=4) as opool,
        ):
            # ---- load params (one packed DMA; engines use views) ----
            pk_sb = spool.tile([128, 384], F32)
            nc.sync.dma_start(out=pk_sb[:], in_=pk_ext[:, :])
            lt_sb = pk_sb[:, 0:128]
            r_sb = pk_sb[0:N, 128:192]
            rt_sb = pk_sb[0:N, 192:256]
            two_i = pk_sb[0:N, 256:320]
            eye = pk_sb[0:N, 320:384]

            # ---- cheap constants (off the critical path) ----
            ones64 = cpool.tile([N, N], F32)
            nc.vector.memset(ones64[:], 1.0)
            at128 = cpool.tile([128, 128], F16)
            nc.vector.memset(at128[:], 0.0)

            with (
                tc.tile_pool(name="pro_ps", bufs=3, space="PSUM") as pps,
                tc.tile_pool(name="nw_ps", bufs=2, space="PSUM") as nps,
            ):
                # ---- S = M11 + M22 = L1@L1^T + L2@L2^T  (PSUM accumulation) ----
                s_ps = pps.tile([N, N], F32, tag="pro")
                nc.tensor.matmul(
                    s_ps[:], lhsT=lt_sb[:, 0:N], rhs=lt_sb[:, 0:N],
                    start=True, stop=False,
                )
                nc.tensor.matmul(
                    s_ps[:], lhsT=lt_sb[:, N:128], rhs=lt_sb[:, N:128],
                    start=False, stop=True,
                )

                # ---- M21 = L1 @ L2^T (the 1/2 of M21h folds into the At copy) ----
                m21_ps = pps.tile([N, N], F32, tag="pro")
                nc.tensor.matmul(
                    m21_ps[:], lhsT=lt_sb[:, 0:N], rhs=lt_sb[:, N:128],
                    start=True, stop=True,
                )

                # ---- whole fp16 input shard -> SBUF; issued here so the 8
                # SP-queue DMA issues (~0.7us each) don't sit ahead of the
                # chain's pool/semaphore setup ----
                xin = []
                for h in range(n_chunks):
                    xt = xpool.tile([128, DMA_COLS], F16, tag=f"xin{h}",
                                    name=f"xin{h}")
                    nc.sync.dma_start(
                        out=xt[:], in_=x_ext[:, h * DMA_COLS : (h + 1) * DMA_COLS]
                    )
                    xin.append(xt)

                # ---- skew = R - R^T;  Bh = S + skew;  Bth = S - skew = Bh^T ----
                skew_sb = spool.tile([N, N], F32)
                nc.vector.tensor_sub(out=skew_sb[:], in0=r_sb, in1=rt_sb)
                bh_sb = spool.tile([N, N], F32)
                nc.vector.tensor_add(out=bh_sb[:], in0=s_ps[:], in1=skew_sb[:])
                bth_sb = spool.tile([N, N], F32)
                nc.vector.tensor_sub(out=bth_sb[:], in0=s_ps[:], in1=skew_sb[:])

                # ---- s = ||Bh||_F^2: ACT square w/ row-accumulate, then a
                # ones-matmul to reduce over partitions + broadcast ----
                sq_sb = spool.tile([N, N], F32)
                nc.vector.tensor_mul(out=sq_sb[:], in0=bh_sb[:], in1=bh_sb[:])
                rs_sb = spool.tile([N, 1], F32)
                nc.vector.reduce_sum(rs_sb[:], sq_sb[:], axis=mybir.AxisListType.X)
                fro_ps = pps.tile([N, 1], F32, tag="pro")
                nc.tensor.matmul(fro_ps[:], lhsT=ones64[:], rhs=rs_sb[:],
                                 start=True, stop=True)
                rcp_sb = spool.tile([N, 1], F32)
                nc.vector.reciprocal(out=rcp_sb[:], in_=fro_ps[:])

                # m21 out of PSUM (off the critical path, ACT engine)
                m21_sb = spool.tile([N, N], F32)
                nc.scalar.copy(out=m21_sb[:], in_=m21_ps[:])

                # ---- F0 = E = I - Bh Bh^T / s ----
                p0_ps = nps.tile([N, N], F32, tag="nw")
                nc.tensor.matmul(p0_ps[:], lhsT=bth_sb[:], rhs=bth_sb[:],
                                 start=True, stop=True)
                p0s_sb = spool.tile([N, N], F32)
                nc.vector.tensor_scalar_mul(p0s_sb[:], p0_ps[:], rcp_sb[:])
                f_prev = upool.tile([N, N], F32, tag="f0", name="f0")
                nc.vector.tensor_sub(out=f_prev[:], in0=eye, in1=p0s_sb[:])

                # ---- F-chain F <- F@F with the G_j = I + F_j factors and
                # their product tree interleaved off the critical path ----
                tree_q = []        # ready product nodes awaiting pairing
                li = 0

                def emit_g(f_tile, j):
                    g = upool.tile([N, N], F32, tag=f"g{j}", name=f"g{j}")
                    nc.vector.tensor_add(out=g[:], in0=eye, in1=f_tile[:])
                    tree_q.append(g)

                def emit_tree_mm(budget):
                    nonlocal li
                    done = 0
                    while done < budget and len(tree_q) >= 2:
                        a = tree_q.pop(0)
                        b = tree_q.pop(0)
                        t_ps = pps.tile([N, N], F32, tag="pro")
                        nc.tensor.matmul(t_ps[:], lhsT=a[:], rhs=b[:],
                                         start=True, stop=True)
                        t_sb = upool.tile([N, N], F32, tag=f"h{li}",
                                          name=f"h{li}")
                        nc.scalar.copy(out=t_sb[:], in_=t_ps[:])
                        li += 1
                        tree_q.append(t_sb)
                        done += 1

                emit_g(f_prev, 0)
                for j in range(1, N_SQUARE):
                    f2_ps = nps.tile([N, N], F32, tag="nw")
                    nc.tensor.matmul(f2_ps[:], lhsT=f_prev[:], rhs=f_prev[:],
                                     start=True, stop=True)
                    f_j = upool.tile([N, N], F32, tag=f"f{j}", name=f"f{j}")
                    nc.vector.tensor_copy(out=f_j[:], in_=f2_ps[:])
                    if j < N_SQUARE - 1:
                        emit_g(f_j, j)
                        emit_tree_mm(1)
                    f_prev = f_j
                # product of G_0..G_19, then W = that @ G_20
                emit_tree_mm(N_SQUARE)
                assert len(tree_q) == 1
                w_pre = tree_q.pop()
                glast = upool.tile([N, N], F32, tag="glast", name="glast")
                nc.vector.tensor_add(out=glast[:], in0=eye, in1=f_prev[:])
                g_last_ps = pps.tile([N, N], F32, tag="pro")
                nc.tensor.matmul(g_last_ps[:], lhsT=w_pre[:], rhs=glast[:],
                                 start=True, stop=True)
                w_sb = upool.tile([N, N], F32, tag="w", name="w")
                nc.vector.tensor_copy(out=w_sb[:], in_=g_last_ps[:])

                # ---- X = (Bh^T W)/s,  X^T = (W Bh)/s ----
                xx_ps = nps.tile([N, N], F32, tag="nw")
                nc.tensor.matmul(xx_ps[:], lhsT=bh_sb[:], rhs=w_sb[:],
                                 start=True, stop=True)
                xxt_ps = nps.tile([N, N], F32, tag="nw")
                nc.tensor.matmul(xxt_ps[:], lhsT=w_sb[:], rhs=bh_sb[:],
                                 start=True, stop=True)
                x_sb = upool.tile([N, N], F32, tag="x", name="x")
                nc.vector.tensor_scalar_mul(x_sb[:], xx_ps[:], rcp_sb[:])
                xt_sb = upool.tile([N, N], F32, tag="xt", name="xt")
                nc.vector.tensor_scalar_mul(xt_sb[:], xxt_ps[:], rcp_sb[:])

                # ---- Newton polish: X <- X(2I - Bh X) (self-correcting) ----
                p_ps = nps.tile([N, N], F32, tag="nw")
                nc.tensor.matmul(p_ps[:], lhsT=bth_sb[:], rhs=x_sb[:],
                                 start=True, stop=True)
                q_sb = spool.tile([N, N], F32, tag="q")
                nc.vector.tensor_sub(out=q_sb[:], in0=two_i, in1=p_ps[:])
                wp = nps.tile([128, 512], F32, tag="nw", name="warm_q")
                nc.tensor.matmul(wp[:], lhsT=xin[0][:, 0:128],
                                 rhs=xin[0][:, 0:512], start=True, stop=True)
                xnt_ps = nps.tile([N, N], F32, tag="nw")
                nc.tensor.matmul(xnt_ps[:], lhsT=q_sb[:], rhs=xt_sb[:],
                                 start=True, stop=True)
                # fold the 1/2 of M21h into the polished X^T so the At
                # copies below are plain (unscaled) copies
                xt2_sb = upool.tile([N, N], F32, tag="xt2", name="xt2")
                nc.vector.tensor_scalar_mul(xt2_sb[:], xnt_ps[:], 0.5)
                wp2 = nps.tile([128, 512], F32, tag="nw", name="warm_x")
                nc.tensor.matmul(wp2[:], lhsT=xin[0][:, 0:128],
                                 rhs=xin[0][:, 0:512], start=True, stop=True)

                # ---- At = 0.5 * M21^T @ X^T, produced in BOTH partition
                # halves (two matmuls, different output partition offsets) so
                # the blockdiag build needs no cross-partition move ----
                at_psa = nps.tile([N, N], F32, tag="nw")
                nc.tensor.matmul(at_psa[:], lhsT=m21_sb[:], rhs=xt2_sb[:],
                                 start=True, stop=True)
                at_psb = nps.tile([128, N], F32, tag="nw")
                nc.tensor.matmul(at_psb[N:128, 0:N], lhsT=m21_sb[:],
                                 rhs=xt2_sb[:], start=True, stop=True)
                nc.vector.tensor_copy(out=at128[0:N, 0:N], in_=at_psa[:])
                nc.scalar.copy(out=at128[N:128, N:128], in_=at_psb[N:128, 0:N])
                # keep the PE busy across the copy gap so the stream starts
                # with the clock ramp already in progress (results discarded)
                for wi in range(4):
                    warm_ps = nps.tile([128, 512], F32, tag="nw",
                                       name=f"warm{wi}")
                    nc.tensor.matmul(warm_ps[:], lhsT=xin[0][:, 0:128],
                                     rhs=xin[0][:, 0:512], start=True,
                                     stop=True)

            # ---- streaming matmul: out = blockdiag(At)^T @ x_shard ----
            # chain PSUM pools are released; the stream gets all 8 banks as
            # four (128, 1024) tiles.  The 1024-col copy latency (~1.2 us)
            # stays inside the 4-tile PSUM runway, so the PE never stalls
            # and ramps to full clock; copies alternate vector/scalar.
            PS_COLS = 1024
            with tc.tile_pool(name="mm_ps", bufs=4, space="PSUM") as mps:
                n_tiles = t_half // PS_COLS
                yout = None
                for i in range(n_tiles):
                    obase = i * PS_COLS
                    ps = mps.tile([128, PS_COLS], F32, tag="mm")
                    for j in range(PS_COLS // MM_COLS):
                        col = obase + j * MM_COLS
                        xt = xin[col // DMA_COLS]
                        off = col % DMA_COLS
                        nc.tensor.matmul(
                            ps[:, j * MM_COLS : (j + 1) * MM_COLS],
                            lhsT=at128[:],
                            rhs=xt[:, off : off + MM_COLS],
                            start=True,
                            stop=True,
                        )
                    if i % 2 == 0:
                        yout = opool.tile([128, OUT_COLS], F16, tag="yout",
                                          name="yout")
                    dst = yout[:, (i % 2) * PS_COLS : (i % 2 + 1) * PS_COLS]
                    if i % 2 == 0:
                        nc.scalar.copy(out=dst, in_=ps[:])
                    else:
                        nc.vector.tensor_copy(out=dst, in_=ps[:])
                    if i % 2 == 1:
                        nc.sync.dma_start(
                            out=out_ext[:, obase + PS_COLS - OUT_COLS : obase + PS_COLS],
                            in_=yout[:],
                        )

    return nc


_NC_CACHE = {}
LAST_PROFILE = None


def _get_nc(t_half=T_HALF):
    if t_half not in _NC_CACHE:
        nc = build_kernel(t_half)
        nc.finalize()  # Bacc: reg alloc + event-semaphore wait splitting
        _NC_CACHE[t_half] = nc
    return _NC_CACHE[t_half]


def _ensure_ntff_hook():
    """The agent image's `antenv` lacks the `axon_hooks` shim that
    `trn_agent_boot` uses to register the NTFF profiling hook (boot
    degrades silently).  Provide the shim and register the hook so
    run_bass_kernel_spmd(trace=True) can capture neuron-profile data."""
    import types

    try:
        from antenv.axon_hooks import get_axon_ntff_profile_hook  # noqa: F401
        return True
    except ImportError:
        pass
    try:
        import antenv
        from trn_agent_boot.trn_boot import _ntff_profile_via_ctypes

        mod = types.ModuleType("antenv.axon_hooks")
        _store = {"h": None}
        mod.set_axon_ntff_profile_hook = lambda h: _store.__setitem__("h", h)
        mod.get_axon_ntff_profile_hook = lambda: _store["h"]
        sys.modules["antenv.axon_hooks"] = mod
        antenv.axon_hooks = mod
        hook = _ntff_profile_via_ctypes("/opt/axon/libaxon_pjrt.so")
        mod.set_axon_ntff_profile_hook(hook)
        return hook is not None
    except Exception as e:  # degrade to no-trace
        print(f"kernel.py: NTFF hook setup failed ({type(e).__name__}: {e})")
        return False


def kernel(x, L, R):
    global LAST_PROFILE
    x = np.ascontiguousarray(np.asarray(x, dtype=np.float32))
    L = np.ascontiguousarray(np.asarray(L, dtype=np.float32))
    R = np.ascontiguousarray(np.asarray(R, dtype=np.float32))
    assert x.shape == (T_FULL, N), x.shape

    X = x.reshape(N, T_FULL)  # row-major view, no copy
    pk = np.zeros((128, 384), dtype=np.float32)
    pk[:, 0:128] = L.T
    pk[0:N, 128:192] = R
    pk[0:N, 192:256] = R.T
    pk[0:N, 256:320] = 2.0 * np.eye(N)
    pk[0:N, 320:384] = np.eye(N)

    in_maps = []
    for c in range(N_CORES):
        shard = np.empty((128, T_HALF), dtype=np.float16)
        base = c * T_CORE
        shard[:N] = X[:, base : base + T_HALF]
        shard[N:] = X[:, base + T_HALF : base + T_CORE]
        in_maps.append({"x": shard, "PK": pk})

    nc = _get_nc()
    trace = os.environ.get("KERNEL_TRACE", "0") == "1"
    if trace:
        trace = _ensure_ntff_hook()
    try:
        res = run_bass_kernel_spmd(
            nc, in_maps, core_ids=list(range(N_CORES)), trace=trace
        )
    except Exception:
        if not trace:
            raise
        print("kernel.py: traced run failed; retrying without trace")
        res = run_bass_kernel_spmd(
            nc, in_maps, core_ids=list(range(N_CORES)), trace=False
        )
    LAST_PROFILE = res

    Y = np.empty((N, T_FULL), dtype=np.float32)
    for c in range(N_CORES):
        o = res.results[c]["out"]
        base = c * T_CORE
        Y[:, base : base + T_HALF] = o[:N]
        Y[:, base + T_HALF : base + T_CORE] = o[N:]
    return Y.reshape(T_FULL, N)
